# revision 1
# baseline (speedup 1.0000x reference)
"""BiModalAttention Trainium2 kernel (v2).

Full-input contract: kernel(mode1, mode2) -> [S, B, 2D] float32.
mode1/mode2: [S=1024, B=32, D=1024] float32.

Reference computation per batch b (m1 = mode1[:, b, :], m2 = mode2[:, b, :]):
    C1 = m1 @ m2.T                  # [S, S]
    a1 = softmax_rows(C1) @ m2 * m1
    a2 = softmax_rows(C1.T) @ m1 * m2
    out[:, b, :] = concat([a1, a2], -1)

Sharding: batch dim across 8 NeuronCores (4 batch elements per core).

Per-core structure (per batch element):
  A. C1 = m1T.T @ m2T in fp32r (d-major layout via casting DMAs). fp32r
     runs as a single fp32_mode=HIGH pass (~2 cyc/row) with a mandatory
     per-matmul weight reload, so matmul count is what matters: C2 = C1.T is
     produced by PE transposes of the C1 strips (4x cheaper than a second
     fp32r matmul). C1 evacuated on ScalarE; negated row-max rm1 on VectorE.
  B. rm1 broadcast across partitions (RM1B[t,s] = -rm1[s]): DVE free-dim
     broadcast of the [P,1] column + PE transpose.
  C. C2 PSUM groups: negated row-max partials (pre-shift) -> rm2; evacuation
     fused with "+(-rm1[s])" on DVE -> epre; ACT exp -> E1T strips (bf16).
  D. E2T = exp(C1 + (-rm2[t] broadcast)) via DVE add + ACT exp -> bf16.
  E. Softmax denominators without extra matmuls: Z1[s] / Z2[t] via ACT
     exp-accumulate passes over the C1 / raw-C2 strips with the per-partition
     negated row-max as bias (all exponents <= 0, so no overflow; a
     factorized exp(rm1-rm2) trick overflows fp32 on this data).
  F. AV matmuls in bf16, 512-wide d-chunks: o1 = E1T.T @ m2chunk,
     o2 = E2T.T @ m1chunk. Evacuation fused as one DVE scalar_tensor_tensor:
     out = (psum * (1/Z)[part]) * gate, gating against the bf16 chunk of the
     other modality (same tile that feeds the AV matmul).
"""

import os
os.environ.setdefault("NEURON_RT_RESET_CORES", "1")
import time

import numpy as np

import concourse.bacc as bacc
import concourse.mybir as mybir
import concourse.tile as tile
from concourse.masks import make_identity
from concourse.bass_utils import run_bass_kernel_spmd

S = 1024
D = 1024
B = 32
N_CORES = 8
BPC = B // N_CORES          # batch elements per core
P = 128                     # partitions
NK = S // P                 # contraction tiles (8)
NI = S // P                 # s tiles (8)
CW = 512                    # AV d-chunk width (bf16 matmul moving dim)
NCH = D // CW               # AV chunks (2)

f32 = mybir.dt.float32
f32r = mybir.dt.float32r
bf16 = mybir.dt.bfloat16
AX = mybir.AxisListType
ALU = mybir.AluOpType
ACTF = mybir.ActivationFunctionType


def _emit_p1(nc, sb, ps, ident, st, j, m1t, m2t):
    # ---- Phase 1: C1 scores (fp32r) ----
    m1t_sb = sb.tile([P, NK, S], f32r, tag="m1t", bufs=1, name=f"m1t_sb{j}")
    m2t_sb = sb.tile([P, NK, S], f32r, tag="m2t", bufs=1, name=f"m2t_sb{j}")
    # halved loads: the C1 k-loop can start on the first half while the
    # second half is still in flight
    for (lo, hi) in ((0, NK // 2), (NK // 2, NK)):
        nc.gpsimd.dma_start(
            out=m1t_sb[:, lo:hi, :],
            in_=m1t[j].rearrange("(k p) s -> p k s", p=P)[:, lo:hi, :])
        nc.gpsimd.dma_start(
            out=m2t_sb[:, lo:hi, :],
            in_=m2t[j].rearrange("(k p) s -> p k s", p=P)[:, lo:hi, :])

    c1 = st["c1"] = []
    rm1 = st["rm1"] = sb.tile([P, NI], f32, tag="rm1", bufs=2, name=f"rm1_{j}")
    for i in range(NI):
        c1_i = sb.tile([P, S], f32, tag="c1", bufs=NI, name=f"c1_{j}_{i}")
        c1.append(c1_i)
        for n in range(2):
            pc = ps.tile([P, 512], f32, tag="c", bufs=4, name=f"pc{j}_{i}_{n}")
            for k in range(NK):
                nc.tensor.matmul(
                    pc,
                    m1t_sb[:, k, i * P:(i + 1) * P],
                    m2t_sb[:, k, n * 512:(n + 1) * 512],
                    start=(k == 0),
                    stop=(k == NK - 1),
                )
            nc.scalar.copy(out=c1_i[:, n * 512:(n + 1) * 512], in_=pc)
        nc.vector.tensor_reduce(rm1[:, i:i + 1], c1_i, axis=AX.X,
                                op=ALU.max, negate=True)


def _keeper(nc, ps, kc, nm):
    # tiny discarded fp32r matmul: keeps the PE HAM activity window busy so
    # the clock gate stays at 8/8 through transpose/softmax phases
    pk = ps.tile([P, 512], f32, tag="av", bufs=4, name=nm)
    nc.tensor.matmul(pk, kc[:, 0:P], kc, start=True, stop=True)


def _emit_p2(nc, sb, ps, ident, kc, st, j):
    c1 = st["c1"]
    rm1 = st["rm1"]

    # ---- negated row-max partition broadcasts ----
    def _bcast_rows(rm_cols, tag, nm):
        rmb = sb.tile([P, S], f32, tag=tag, bufs=1, name=nm)
        for g in range(2):
            pt = ps.tile([P, 512], f32, tag="c", bufs=4, name=f"{nm}_pt{g}")
            for q in range(4):
                i = g * 4 + q
                xb = sb.tile([P, P], f32, tag="xb", bufs=1, name=f"{nm}_xb{i}")
                nc.vector.tensor_copy(xb, rm_cols[:, i:i + 1].broadcast_to([P, P]))
                nc.tensor.transpose(pt[:, q * P:(q + 1) * P], xb, ident)
            nc.scalar.copy(out=rmb[:, g * 512:(g + 1) * 512], in_=pt)
        return rmb

    rm1b = _bcast_rows(rm1, "rm1b", f"rm1b_{j}")

    # ---- C2 strips via PE transpose -> rm2, Z2, E1T = exp(C2 - rm1[s]) ----
    e1 = st["e1"] = []
    rm2p = sb.tile([P, 2 * NK], f32, tag="rm2p", bufs=2, name=f"rm2p_{j}")
    rm2 = sb.tile([P, NK], f32, tag="rm2", bufs=2, name=f"rm2_{j}")
    z2p = sb.tile([P, 2 * NK], f32, tag="z2p", bufs=2, name=f"z2p_{j}")
    z2 = sb.tile([P, NK], f32, tag="z2", bufs=2, name=f"z2_{j}")
    for t in range(NK):
        e1_t = sb.tile([P, S], bf16, tag="e1", bufs=NK + 2, name=f"e1_{j}_{t}")
        e1.append(e1_t)
        epre = sb.tile([P, S], f32, tag="h", bufs=2, name=f"epre1_{j}_{t}")
        pts = []
        for g in range(2):
            pt = ps.tile([P, 512], f32, tag="c", bufs=4, name=f"pc2_{j}_{t}_{g}")
            pts.append(pt)
            for q in range(4):
                i = g * 4 + q
                nc.tensor.transpose(pt[:, q * P:(q + 1) * P],
                                    c1[i][:, t * P:(t + 1) * P], ident)
            nc.vector.tensor_reduce(rm2p[:, 2 * t + g:2 * t + g + 1], pt,
                                    axis=AX.X, op=ALU.max, negate=True)
        nc.vector.tensor_tensor(rm2[:, t:t + 1], rm2p[:, 2 * t:2 * t + 1],
                                rm2p[:, 2 * t + 1:2 * t + 2], op=ALU.min)
        for g in range(2):
            # Z2 partial straight from PSUM; fused shift on evacuation
            scrz = sb.tile([P, 512], bf16, tag="scr", bufs=2, name=f"scrz_{j}_{t}_{g}")
            nc.scalar.activation(scrz, pts[g], ACTF.Exp, bias=rm2[:, t:t + 1],
                                 accum_out=z2p[:, 2 * t + g:2 * t + g + 1])
            nc.vector.tensor_add(epre[:, g * 512:(g + 1) * 512], pts[g],
                                 rm1b[:, g * 512:(g + 1) * 512])
        nc.vector.tensor_tensor(z2[:, t:t + 1], z2p[:, 2 * t:2 * t + 1],
                                z2p[:, 2 * t + 1:2 * t + 2], op=ALU.add)
        nc.scalar.activation(e1_t, epre, ACTF.Exp)
        _keeper(nc, ps, kc, f"kp1_{j}_{t}")

    rm2b = _bcast_rows(rm2, "rm2b", f"rm2b_{j}")

    # ---- E2T = exp(C1 - rm2[t]) + Z1 ----
    z1 = sb.tile([P, NI], f32, tag="z1", bufs=2, name=f"z1_{j}")
    e2 = st["e2"] = []
    for i in range(NI):
        e2_i = sb.tile([P, S], bf16, tag="e2", bufs=NI + 2, name=f"e2_{j}_{i}")
        e2.append(e2_i)
        epre2 = sb.tile([P, S], f32, tag="epre", bufs=2, name=f"epre2_{j}_{i}")
        nc.vector.tensor_add(epre2, c1[i], rm2b)
        nc.scalar.activation(e2_i, epre2, ACTF.Exp)
        # Z1[s] = sum_t exp(C1[s,t] - rm1[s]): ACT pass, output discarded
        scr = sb.tile([P, S], bf16, tag="scr", bufs=2, name=f"scr1_{j}_{i}")
        nc.scalar.activation(scr, c1[i], ACTF.Exp, bias=rm1[:, i:i + 1],
                             accum_out=z1[:, i:i + 1])
        _keeper(nc, ps, kc, f"kp2_{j}_{i}")

    invz1 = st["invz1"] = sb.tile([P, NI], f32, tag="invz1", bufs=2, name=f"invz1_{j}")
    invz2 = st["invz2"] = sb.tile([P, NI], f32, tag="invz2", bufs=2, name=f"invz2_{j}")
    nc.vector.reciprocal(invz1, z1)
    nc.vector.reciprocal(invz2, z2)


def _emit_p3(nc, sb, ps, st, j, m1n, m2n, outp):
    e1, e2 = st["e1"], st["e2"]
    invz1, invz2 = st["invz1"], st["invz2"]
    for c in range(NCH):
        c0 = c * CW
        r2 = sb.tile([P, NK, CW], bf16, tag="rhs", bufs=3, name=f"r2_{j}_{c}")
        r1 = sb.tile([P, NK, CW], bf16, tag="rhs", bufs=3, name=f"r1_{j}_{c}")
        nc.gpsimd.dma_start(
            out=r2, in_=m2n[j].rearrange("(k p) d -> p k d", p=P)[:, :, c0:c0 + CW])
        nc.gpsimd.dma_start(
            out=r1, in_=m1n[j].rearrange("(k p) d -> p k d", p=P)[:, :, c0:c0 + CW])

        for i in range(NI):
            for (es, rhs, gate, invz, dbase) in (
                (e1, r2, r1, invz1, 0),
                (e2, r1, r2, invz2, D),
            ):
                pav = ps.tile([P, CW], f32, tag="av", bufs=4,
                              name=f"pav{j}_{c}_{i}_{dbase}")
                for k in range(NK):
                    nc.tensor.matmul(
                        pav,
                        es[k][:, i * P:(i + 1) * P],
                        rhs[:, k, :],
                        start=(k == 0),
                        stop=(k == NK - 1),
                    )
                a_sb = sb.tile([P, CW], f32, tag="ao", bufs=4,
                               name=f"a{j}_{c}_{i}_{dbase}")
                nc.vector.scalar_tensor_tensor(
                    a_sb, pav, invz[:, i:i + 1],
                    gate[:, i, :],
                    op0=ALU.mult, op1=ALU.mult)
                nc.sync.dma_start(
                    out=outp[j, i * P:(i + 1) * P,
                             dbase + c0:dbase + c0 + CW],
                    in_=a_sb)


def _build():
    nc = bacc.Bacc("TRN2", target_bir_lowering=False, debug=False,
                   num_devices=N_CORES)
    m1n = nc.dram_tensor("m1n", [BPC, S, D], f32, kind="ExternalInput").ap()
    m2n = nc.dram_tensor("m2n", [BPC, S, D], f32, kind="ExternalInput").ap()
    m1t = nc.dram_tensor("m1t", [BPC, D, S], f32, kind="ExternalInput").ap()
    m2t = nc.dram_tensor("m2t", [BPC, D, S], f32, kind="ExternalInput").ap()
    outp = nc.dram_tensor("out", [BPC, S, 2 * D], f32, kind="ExternalOutput").ap()

    with tile.TileContext(nc) as tc:
        with tc.tile_pool(name="consts", bufs=1) as consts, \
             tc.tile_pool(name="sb", bufs=1) as sb, \
             tc.tile_pool(name="ps", bufs=1, space="PSUM") as ps:
            ident = consts.tile([P, P], f32)
            make_identity(nc, ident)
            kc = consts.tile([P, 512], f32r)
            nc.vector.memset(kc.bitcast(f32), 1.0)
            # Software-pipelined emission: PE stream becomes
            # C1(0), trans(0), C1(1), AV(0), trans(1), C1(2), AV(1), ...
            # so scores matmuls of batch j+1 fill the PE while batch j's
            # softmax runs on Vector/Scalar, and HAM stays warm. P1(j+1)
            # must be emitted after P2(j): the c1 strip slots are freed by
            # P2(j) work that sits behind P1(j+1) in the per-engine queues
            # otherwise (head-of-line deadlock).
            sts = [dict() for _ in range(BPC)]
            _emit_p1(nc, sb, ps, ident, sts[0], 0, m1t, m2t)
            for j in range(BPC):
                _emit_p2(nc, sb, ps, ident, kc, sts[j], j)
                if j + 1 < BPC:
                    _emit_p1(nc, sb, ps, ident, sts[j + 1], j + 1, m1t, m2t)
                _emit_p3(nc, sb, ps, sts[j], j, m1n, m2n, outp)
    nc.compile()
    return nc


_NC_CACHE = None


def _get_nc():
    global _NC_CACHE
    if _NC_CACHE is None:
        _NC_CACHE = _build()
    return _NC_CACHE


def kernel(mode1: np.ndarray, mode2: np.ndarray, _trace: bool = False,
           _result_box: dict | None = None) -> np.ndarray:
    mode1 = np.asarray(mode1, dtype=np.float32)
    mode2 = np.asarray(mode2, dtype=np.float32)

    m1n_all = np.ascontiguousarray(mode1.transpose(1, 0, 2))  # [B, S, D]
    m2n_all = np.ascontiguousarray(mode2.transpose(1, 0, 2))
    m1t_all = np.ascontiguousarray(mode1.transpose(1, 2, 0))  # [B, D, S]
    m2t_all = np.ascontiguousarray(mode2.transpose(1, 2, 0))

    nc = _get_nc()
    in_maps = []
    for c in range(N_CORES):
        lo, hi = c * BPC, (c + 1) * BPC
        in_maps.append({
            "m1n": m1n_all[lo:hi],
            "m2n": m2n_all[lo:hi],
            "m1t": m1t_all[lo:hi],
            "m2t": m2t_all[lo:hi],
        })

    r = None
    last_err = None
    for attempt in range(3):
        try:
            r = run_bass_kernel_spmd(nc, in_maps, list(range(N_CORES)),
                                     trace=_trace)
            break
        except Exception as e:  # transient NRT exec-unit errors recover on retry
            last_err = e
            time.sleep(2.0)
    if r is None:
        raise last_err
    if _result_box is not None:
        _result_box["result"] = r

    out = np.empty((S, B, 2 * D), dtype=np.float32)
    for c in range(N_CORES):
        res = r.results[c]["out"]  # [BPC, S, 2D]
        out[:, c * BPC:(c + 1) * BPC, :] = res.transpose(1, 0, 2)
    return out



# revision 9
# speedup vs baseline: 1.2077x; 1.2077x over previous
"""BiModalAttention Trainium2 kernel (v3).

Full-input contract: kernel(mode1, mode2) -> [S, B, 2D] float32.
mode1/mode2: [S=1024, B=32, D=1024] float32.

Reference computation per batch b (m1 = mode1[:, b, :], m2 = mode2[:, b, :]):
    C1 = m1 @ m2.T                  # [S, S]
    a1 = softmax_rows(C1) @ m2 * m1
    a2 = softmax_rows(C1.T) @ m1 * m2
    out[:, b, :] = concat([a1, a2], -1)

Sharding: batch dim across 8 NeuronCores (4 batch elements per core).

v3 design notes (changes vs v2, driven by the v2 ntff profile):
  * v2 lost ~114us to PE idle gaps and ~56us-equivalent to HAM 4/8-duty
    epochs that follow gap-containing epochs. The fix is a denser software
    pipeline: the transpose/softmax phase of batch j (phase A) is emitted
    interleaved at instruction granularity with the AV matmuls of batch j-1,
    and the scores phase of batch j+1 (phase CP) is interleaved with the
    exp(E2) build of batch j. The PE stream then has no dependency-gated
    stretches and the keeper matmuls of v2 are unnecessary.
  * DVE was ~47% busy (reductions + adds + AV evacuation); the Pool engine
    was idle. The softmax pre-shift adds (epre/epre2) moved to Pool
    (nc.gpsimd), in bf16 (stores pre-exponent values; exp(bf16(x)) only
    perturbs weights O(0.2%) multiplicatively, verified 3.5e-3 scale-rel).
  * All input DMA issues stay on the Pool queue but are emitted exactly at
    the point where their WAR wait is already satisfied (staggered
    prefetch), so they never head-of-line-block Pool compute. Out stores on
    sync. First batch's score operands load in need-order so C1(0) starts
    after ~2.5MB instead of 8MB.
  * PSUM: pc(2) + pt(4) + pav(2) = 8 banks exactly.

Phase structure per core (j = batch index, 4 per core):
  CP(j): for i: [Pool epre2(j-1,i); ACT exp e2(j-1,i)] + [C1(j) block i
         (16 fp32r matmuls, ACT evac, DVE row-max)]  -- PE: 29us dense
  A(j):  rm1b transposes; 8 sections t: [C2 strip transposes (PE), DVE
         col-max reduce, ACT exp+Z2-accum, Pool epre add, ACT exp E1T]
         + 4 AV(j-1) groups (PE) + ACT Z1 pass i=t; rm2b; 1/Z  -- PE: ~66us
  AV groups: [c0 dir1 i0-7][c1 dir1][c0 dir2][c1 dir2]; evac fused
         (psum * invZ[part]) * gate on DVE; stores on sync.
"""

import os
os.environ.setdefault("NEURON_RT_RESET_CORES", "1")
import time

import numpy as np

import concourse.bacc as bacc
import concourse.mybir as mybir
import concourse.tile as tile
from concourse.masks import make_identity
from concourse.bass_utils import run_bass_kernel_spmd

S = 1024
D = 1024
B = 32
N_CORES = 8
BPC = B // N_CORES          # batch elements per core
P = 128                     # partitions
NK = S // P                 # contraction tiles (8)
NI = S // P                 # s tiles (8)
CW = 512                    # AV d-chunk width
NCH = D // CW               # AV chunks (2)

f32 = mybir.dt.float32
f32r = mybir.dt.float32r
bf16 = mybir.dt.bfloat16
AX = mybir.AxisListType
ALU = mybir.AluOpType
ACTF = mybir.ActivationFunctionType

E1_BUFS = 12   # e1(j+1)_t reuses e1(j)_(t-4): free after AV(j) dir1 (section 3)
E2_BUFS = 8    # e2(j+1)_i reuses e2(j)_i: AV(j) dir2 done before CP(j+2)
RHS_BUFS = 5   # rotation verified against staggered prefetch points
C1_BUFS = 8


def _emit_m_loads(nc, sb, st, j, m1t, m2t, head):
    """Score operands in d-major [d_part, k, s] layout (f32 bits as f32r)."""
    m1t_sb = sb.tile([P, NK, S], f32r, tag="m1t", bufs=1, name=f"m1t{j}")
    m2t_sb = sb.tile([P, NK, S], f32r, tag="m2t", bufs=1, name=f"m2t{j}")
    st["m1t"], st["m2t"] = m1t_sb, m2t_sb
    a1 = m1t[j].rearrange("(k p) s -> p k s", p=P)
    a2 = m2t[j].rearrange("(k p) s -> p k s", p=P)
    if head:
        # batch 0 runs C1 with n outer / i inner; deliver in need-order
        nc.gpsimd.dma_start(out=m1t_sb[:, :, 0:P], in_=a1[:, :, 0:P])
        nc.gpsimd.dma_start(out=m2t_sb[:, :, 0:CW], in_=a2[:, :, 0:CW])
        nc.gpsimd.dma_start(out=m1t_sb[:, :, P:CW], in_=a1[:, :, P:CW])
        nc.gpsimd.dma_start(out=m1t_sb[:, :, CW:S], in_=a1[:, :, CW:S])
        nc.gpsimd.dma_start(out=m2t_sb[:, :, CW:S], in_=a2[:, :, CW:S])
    else:
        nc.gpsimd.dma_start(out=m1t_sb, in_=a1)
        nc.gpsimd.dma_start(out=m2t_sb, in_=a2)


# r-chunk index -> (modality, c): AV rhs/gate tiles, natural [t_part, k, d]
_R_KEYS = (("r2", 0), ("r1", 0), ("r2", 1), ("r1", 1))


def _emit_r_load(nc, sb, st, j, which, m1n, m2n):
    key = _R_KEYS[which]
    mn = m1n if key[0] == "r1" else m2n
    c = key[1]
    t = sb.tile([P, NK, CW], bf16, tag="rhs", bufs=RHS_BUFS,
                name=f"r{j}_{which}")
    nc.gpsimd.dma_start(
        out=t,
        in_=mn[j].rearrange("(k p) d -> p k d", p=P)[:, :, c * CW:(c + 1) * CW])
    st[key] = t


def _emit_c1_block(nc, sb, ps, st, j, i, n_list):
    m1t_sb, m2t_sb = st["m1t"], st["m2t"]
    c1 = st.setdefault("c1", {})
    if i not in c1:
        c1[i] = sb.tile([P, S], f32, tag="c1", bufs=C1_BUFS, name=f"c1_{j}_{i}")
    for n in n_list:
        pc = ps.tile([P, CW], f32, tag="pc", bufs=2, name=f"pc{j}_{i}_{n}")
        for k in range(NK):
            nc.tensor.matmul(
                pc,
                m1t_sb[:, k, i * P:(i + 1) * P],
                m2t_sb[:, k, n * CW:(n + 1) * CW],
                start=(k == 0),
                stop=(k == NK - 1),
            )
        nc.scalar.copy(out=c1[i][:, n * CW:(n + 1) * CW], in_=pc)


def _emit_rm1_reduce(nc, sb, st, j, i):
    if "rm1" not in st:
        st["rm1"] = sb.tile([P, NI], f32, tag="rm1", bufs=2, name=f"rm1_{j}")
    nc.vector.tensor_reduce(st["rm1"][:, i:i + 1], st["c1"][i], axis=AX.X,
                            op=ALU.max, negate=True)


def _emit_rmb(nc, sb, ps, ident, st, j, which):
    """Broadcast negated row-max across partitions: rmb[t, s] = rm[s]."""
    rm = st[which]
    rmb = sb.tile([P, S], f32, tag=which + "b", bufs=1, name=f"{which}b_{j}")
    for g in range(2):
        pt = ps.tile([P, CW], f32, tag="pt", bufs=4, name=f"{which}b_pt{j}_{g}")
        for q in range(4):
            i = g * 4 + q
            xb = sb.tile([P, P], f32, tag="xb", bufs=2,
                         name=f"{which}b_xb{j}_{i}")
            nc.vector.tensor_copy(xb, rm[:, i:i + 1].broadcast_to([P, P]))
            nc.tensor.transpose(pt[:, q * P:(q + 1) * P], xb, ident)
        nc.scalar.copy(out=rmb[:, g * CW:(g + 1) * CW], in_=pt)
    st[which + "b"] = rmb


def _emit_trans_section(nc, sb, ps, ident, st, j, t):
    """C2 strip t: PE transposes -> rm2 partials -> Z2 accum + E1T strip."""
    c1 = st["c1"]
    rm1b = st["rm1b"]
    rm2p, rm2, z2p = st["rm2p"], st["rm2"], st["z2p"]
    e1 = st.setdefault("e1", {})
    e1[t] = sb.tile([P, S], bf16, tag="e1", bufs=E1_BUFS, name=f"e1_{j}_{t}")
    epre = sb.tile([P, S], bf16, tag="ep1", bufs=2, name=f"ep1_{j}_{t}")
    pts = []
    for g in range(2):
        pt = ps.tile([P, CW], f32, tag="pt", bufs=4, name=f"pt{j}_{t}_{g}")
        pts.append(pt)
        for q in range(4):
            i = g * 4 + q
            nc.tensor.transpose(pt[:, q * P:(q + 1) * P],
                                c1[i][:, t * P:(t + 1) * P], ident)
        nc.vector.tensor_reduce(rm2p[:, 2 * t + g:2 * t + g + 1], pts[g],
                                axis=AX.X, op=ALU.max, negate=True)
    nc.vector.tensor_tensor(rm2[:, t:t + 1], rm2p[:, 2 * t:2 * t + 1],
                            rm2p[:, 2 * t + 1:2 * t + 2], op=ALU.min)
    for g in range(2):
        # Z2 partial straight from PSUM; exp output discarded
        scrz = sb.tile([P, CW], bf16, tag="scr", bufs=2,
                       name=f"scrz_{j}_{t}_{g}")
        nc.scalar.activation(scrz, pts[g], ACTF.Exp, bias=rm2[:, t:t + 1],
                             accum_out=z2p[:, 2 * t + g:2 * t + g + 1])
        # PSUM-reading add must stay on DVE (GPSIMD can't access PSUM)
        nc.vector.tensor_add(epre[:, g * CW:(g + 1) * CW], pts[g],
                             rm1b[:, g * CW:(g + 1) * CW])
    nc.scalar.activation(e1[t], epre, ACTF.Exp)


def _emit_scr(nc, sb, st, j, i):
    """Z1[s] accumulation pass over c1 strip i (exp output discarded)."""
    scr = sb.tile([P, S], bf16, tag="scr", bufs=2, name=f"scr_{j}_{i}")
    nc.scalar.activation(scr, st["c1"][i], ACTF.Exp, bias=st["rm1"][:, i:i + 1],
                         accum_out=st["z1"][:, i:i + 1])


def _emit_e2(nc, sb, st, j, i):
    """E2T strip i = exp(C1 - rm2[t]) in [s_part, t] layout (AV stationary)."""
    e2 = st.setdefault("e2", {})
    e2[i] = sb.tile([P, S], bf16, tag="e2", bufs=E2_BUFS, name=f"e2_{j}_{i}")
    epre2 = sb.tile([P, S], bf16, tag="ep2", bufs=2, name=f"ep2_{j}_{i}")
    nc.gpsimd.tensor_add(epre2, st["c1"][i], st["rm2b"])
    nc.scalar.activation(e2[i], epre2, ACTF.Exp)


def _emit_av_group(nc, sb, ps, st, j, c, dirx, i, outp):
    if dirx == 1:
        es, rhs, gate, invz, dbase = (st["e1"], st[("r2", c)], st[("r1", c)],
                                      st["invz1"], 0)
    else:
        es, rhs, gate, invz, dbase = (st["e2"], st[("r1", c)], st[("r2", c)],
                                      st["invz2"], D)
    pav = ps.tile([P, CW], f32, tag="pav", bufs=2, name=f"pav{j}_{c}_{dirx}_{i}")
    for k in range(NK):
        nc.tensor.matmul(
            pav,
            es[k][:, i * P:(i + 1) * P],
            rhs[:, k, :],
            start=(k == 0),
            stop=(k == NK - 1),
        )
    a_sb = sb.tile([P, CW], f32, tag="ao", bufs=4, name=f"a{j}_{c}_{dirx}_{i}")
    nc.vector.scalar_tensor_tensor(a_sb, pav, invz[:, i:i + 1], gate[:, i, :],
                                   op0=ALU.mult, op1=ALU.mult)
    nc.sync.dma_start(
        out=outp[j, i * P:(i + 1) * P, dbase + c * CW:dbase + (c + 1) * CW],
        in_=a_sb)


def _av_group_list():
    return [(c, dirx, i)
            for (c, dirx) in ((0, 1), (1, 1), (0, 2), (1, 2))
            for i in range(NI)]


def _emit_A(nc, sb, ps, ident, sts, j, outp, m1n, m2n, m1t, m2t):
    """Phase A(j): transposes/softmax of batch j + AV of batch j-1,
    interleaved 1 transpose-group : 4 AV-groups per section."""
    st = sts[j]
    st["rm2p"] = sb.tile([P, 2 * NK], f32, tag="rm2p", bufs=2, name=f"rm2p_{j}")
    st["rm2"] = sb.tile([P, NK], f32, tag="rm2", bufs=2, name=f"rm2_{j}")
    st["z2p"] = sb.tile([P, 2 * NK], f32, tag="z2p", bufs=2, name=f"z2p_{j}")
    st["z2"] = sb.tile([P, NK], f32, tag="z2", bufs=2, name=f"z2_{j}")
    st["z1"] = sb.tile([P, NI], f32, tag="z1", bufs=2, name=f"z1_{j}")
    _emit_rmb(nc, sb, ps, ident, st, j, "rm1")
    if j + 1 < BPC:
        # m(j+1) loads: WAR on C1(j) matmuls releases right about now, so
        # this never head-of-line-blocks Pool compute behind it
        _emit_m_loads(nc, sb, sts[j + 1], j + 1, m1t, m2t, head=False)

    av = _av_group_list() if j >= 1 else []
    stp = sts[j - 1] if j >= 1 else None
    gi = 0
    for t in range(NK):
        _emit_trans_section(nc, sb, ps, ident, st, j, t)
        _emit_scr(nc, sb, st, j, t)
        for _ in range(4):
            if gi < len(av):
                c, dirx, i = av[gi]
                _emit_av_group(nc, sb, ps, stp, j - 1, c, dirx, i, outp)
                gi += 1
        if t == 5 and j >= 1:
            # r(j)_1 / r(j)_2 slots freed by AV(j-1) c0-dir2 (section 5)
            _emit_r_load(nc, sb, st, j, 1, m1n, m2n)
            _emit_r_load(nc, sb, st, j, 2, m1n, m2n)
    _emit_rmb(nc, sb, ps, ident, st, j, "rm2")
    for t in range(NK):
        nc.vector.tensor_tensor(st["z2"][:, t:t + 1],
                                st["z2p"][:, 2 * t:2 * t + 1],
                                st["z2p"][:, 2 * t + 1:2 * t + 2], op=ALU.add)
    st["invz1"] = sb.tile([P, NI], f32, tag="invz1", bufs=2, name=f"invz1_{j}")
    st["invz2"] = sb.tile([P, NI], f32, tag="invz2", bufs=2, name=f"invz2_{j}")
    nc.vector.reciprocal(st["invz2"], st["z2"])
    nc.vector.reciprocal(st["invz1"], st["z1"])
    if j >= 1:
        # r(j)_3 slot freed by AV(j-1) c1-dir2 (section 7)
        _emit_r_load(nc, sb, st, j, 3, m1n, m2n)


def _emit_CP(nc, sb, ps, sts, j, m1t, m2t, m1n, m2n):
    """Phase CP(j): C1 scores of batch j + E2T build of batch j-1."""
    st, stp = sts[j], sts[j - 1]
    _emit_r_load(nc, sb, st, j, 0, m1n, m2n)
    for i in range(NI):
        _emit_e2(nc, sb, stp, j - 1, i)
        _emit_c1_block(nc, sb, ps, st, j, i, (0, 1))
        _emit_rm1_reduce(nc, sb, st, j, i)


def _emit_tail(nc, sb, ps, sts, outp):
    """E2T(last) build interleaved with AV(last) dir1; then dir2."""
    j = BPC - 1
    st = sts[j]
    dir1 = [(c, 1, i) for c in (0, 1) for i in range(NI)]
    gi = 0
    for i in range(NI):
        _emit_e2(nc, sb, st, j, i)
        for _ in range(2):
            c, dirx, ii = dir1[gi]
            _emit_av_group(nc, sb, ps, st, j, c, dirx, ii, outp)
            gi += 1
    for c in (0, 1):
        for i in range(NI):
            _emit_av_group(nc, sb, ps, st, j, c, 2, i, outp)


def _build():
    nc = bacc.Bacc("TRN2", target_bir_lowering=False, debug=False,
                   num_devices=N_CORES)
    m1n = nc.dram_tensor("m1n", [BPC, S, D], f32, kind="ExternalInput").ap()
    m2n = nc.dram_tensor("m2n", [BPC, S, D], f32, kind="ExternalInput").ap()
    m1t = nc.dram_tensor("m1t", [BPC, D, S], f32, kind="ExternalInput").ap()
    m2t = nc.dram_tensor("m2t", [BPC, D, S], f32, kind="ExternalInput").ap()
    outp = nc.dram_tensor("out", [BPC, S, 2 * D], f32, kind="ExternalOutput").ap()

    with tile.TileContext(nc) as tc:
        with tc.tile_pool(name="consts", bufs=1) as consts, \
             tc.tile_pool(name="sb", bufs=1) as sb, \
             tc.tile_pool(name="ps", bufs=1, space="PSUM") as ps:
            ident = consts.tile([P, P], f32)
            make_identity(nc, ident)
            sts = [dict() for _ in range(BPC)]

            # head: prioritized loads + r(0) chunks
            _emit_m_loads(nc, sb, sts[0], 0, m1t, m2t, head=True)
            for w in range(4):
                _emit_r_load(nc, sb, sts[0], 0, w, m1n, m2n)

            # CP(0): C1(0), n outer so compute starts after ~2.5MB of DMA
            for n in (0, 1):
                for i in range(NI):
                    _emit_c1_block(nc, sb, ps, sts[0], 0, i, (n,))
                    if n == 1:
                        _emit_rm1_reduce(nc, sb, sts[0], 0, i)

            _emit_A(nc, sb, ps, ident, sts, 0, outp, m1n, m2n, m1t, m2t)
            for j in range(1, BPC):
                _emit_CP(nc, sb, ps, sts, j, m1t, m2t, m1n, m2n)
                _emit_A(nc, sb, ps, ident, sts, j, outp, m1n, m2n, m1t, m2t)
            _emit_tail(nc, sb, ps, sts, outp)
    nc.compile()
    return nc


_NC_CACHE = None


def _get_nc():
    global _NC_CACHE
    if _NC_CACHE is None:
        _NC_CACHE = _build()
    return _NC_CACHE


def kernel(mode1: np.ndarray, mode2: np.ndarray, _trace: bool = False,
           _result_box: dict | None = None) -> np.ndarray:
    mode1 = np.asarray(mode1, dtype=np.float32)
    mode2 = np.asarray(mode2, dtype=np.float32)

    m1n_all = np.ascontiguousarray(mode1.transpose(1, 0, 2))  # [B, S, D]
    m2n_all = np.ascontiguousarray(mode2.transpose(1, 0, 2))
    m1t_all = np.ascontiguousarray(mode1.transpose(1, 2, 0))  # [B, D, S]
    m2t_all = np.ascontiguousarray(mode2.transpose(1, 2, 0))

    nc = _get_nc()
    in_maps = []
    for c in range(N_CORES):
        lo, hi = c * BPC, (c + 1) * BPC
        in_maps.append({
            "m1n": m1n_all[lo:hi],
            "m2n": m2n_all[lo:hi],
            "m1t": m1t_all[lo:hi],
            "m2t": m2t_all[lo:hi],
        })

    r = None
    last_err = None
    for attempt in range(3):
        try:
            r = run_bass_kernel_spmd(nc, in_maps, list(range(N_CORES)),
                                     trace=_trace)
            break
        except Exception as e:  # transient NRT exec-unit errors recover on retry
            last_err = e
            time.sleep(2.0)
    if r is None:
        raise last_err
    if _result_box is not None:
        _result_box["result"] = r

    out = np.empty((S, B, 2 * D), dtype=np.float32)
    for c in range(N_CORES):
        res = r.results[c]["out"]  # [BPC, S, 2D]
        out[:, c * BPC:(c + 1) * BPC, :] = res.transpose(1, 0, 2)
    return out
